# revision 1
# baseline (speedup 1.0000x reference)
"""BoxE scorer kernel for Trainium2 (8 NeuronCores, label-sharded).

Computes out[b,l] = -|| per_dim(x[b], box[l]) ||_2 for
  y: (2048, 256) f32   (per-label box params: mn = y[:, :128], delta = softplus(y[:, 128:]))
  x: (1024, 128) f32
  out: (1024, 2048) f32

Math: with cen = mn + d/2, hd = d/2, l1 = |x - cen|, bb = d+1,
      a = 1/(bb + 1e-10), c = -(d/2)(d - 1/(d+1e-10)):
  per_dim = inside ? l1*a : l1*bb + c        (inside <=> l1 <= hd)
  dist2   = sum_d (l1*a)^2 + s*R,  s = [l1 > hd],
  R = (l1*D + c)(l1*(a+bb) + c),  D = bb - a.
With m = relu(l1 - hd) (and m*s = m exactly):
  sum_d s*R = sum_d alpha*m^2 + beta*m + gamma*s
  alpha = D*(a+bb)
  beta  = 2*alpha*hd + c*(D + a + bb)
  gamma = (hd*D + c)(hd*(a+bb) + c)
  base  = sum_d a^2*x^2 - 2*a^2*cen*x + (a*cen)^2   (3 plain matmuls)

Per-core: 256 labels, full batch. DVE: m, s, m^2 (fp16 planes); ACT: l1;
PE: base matmuls + per-label weighted partition-reductions into PSUM.
"""

import os
from contextlib import ExitStack

import numpy as np

import concourse.bass as bass
import concourse.tile as tile
from concourse import bacc, mybir
from concourse import bass_utils

F32 = mybir.dt.float32
F16 = mybir.dt.float16
BF16 = mybir.dt.bfloat16
A = mybir.AluOpType
ACT = mybir.ActivationFunctionType

B = 1024      # batch
H = 128       # hidden
L = 2048      # num labels
N_CORES = 8
LPC = L // N_CORES   # labels per core
NBCH = B // 128      # batch chunks of 128
GRP = 8              # labels per grouped DVE instruction


def build_nc(repeat: int = 1, ablate: frozenset = frozenset()):
    nc = bacc.Bacc("TRN2", target_bir_lowering=False, debug=False,
                   num_devices=N_CORES)
    xT_d = nc.dram_tensor("xT", (H, B), F32, kind="ExternalInput")
    mnT_d = nc.dram_tensor("mnT", (H, LPC), F32, kind="ExternalInput")
    rawT_d = nc.dram_tensor("rawT", (H, LPC), F32, kind="ExternalInput")
    out_d = nc.dram_tensor("out", (B, LPC), F32, kind="ExternalOutput")

    with tile.TileContext(nc) as tc:
        with ExitStack() as ctx:
            cpool = ctx.enter_context(tc.tile_pool(name="consts", bufs=1))
            pspool = ctx.enter_context(
                tc.tile_pool(name="psum", bufs=1, space=bass.MemorySpace.PSUM))

            # ---- load inputs ----
            ppool_cm = tc.tile_pool(name="pre", bufs=1)
            ppool = ppool_cm.__enter__()
            xT = cpool.tile([H, B], F32, tag="xT")
            nc.sync.dma_start(xT[:], xT_d.ap())
            mnT = ppool.tile([H, LPC], F32, tag="mnT")
            nc.sync.dma_start(mnT[:], mnT_d.ap())
            rawT = ppool.tile([H, LPC], F32, tag="rawT")
            nc.sync.dma_start(rawT[:], rawT_d.ap())

            def f32t(tag, shape=(H, LPC), pool=None):
                return (pool or cpool).tile(list(shape), F32, tag=tag,
                                            name=tag)

            # ---- per-label coefficient precompute (all [H, LPC] f32) ----
            # delta = softplus(raw) = ln(1 + exp(raw))
            e = f32t("e", pool=ppool)
            nc.scalar.activation(e[:], rawT[:], ACT.Exp)
            e1 = f32t("e1", pool=ppool)
            nc.vector.tensor_scalar(e1[:], e[:], 1.0, None, A.add)
            delta = f32t("delta", pool=ppool)
            nc.scalar.activation(delta[:], e1[:], ACT.Ln)

            hd = f32t("hd")                      # d/2 (kept: ts scalars)
            nc.vector.tensor_scalar(hd[:], delta[:], 0.5, None, A.mult)
            cen = f32t("cen")                    # mn + d/2 (kept: ts scalars)
            nc.vector.tensor_tensor(cen[:], mnT[:], hd[:], A.add)
            invhd = f32t("invhd")                # 1/hd (rescale for l1')
            nc.vector.reciprocal(invhd[:], hd[:])
            cod = f32t("cod")                    # cen/hd (DVE l1' path)
            nc.vector.tensor_tensor(cod[:], cen[:], invhd[:], A.mult)
            ncod = f32t("ncod")                  # -cen/hd (ACT l1' bias)
            nc.vector.tensor_scalar(ncod[:], cod[:], -1.0, None, A.mult)

            dp1 = f32t("dp1", pool=ppool)        # bb = d+1
            nc.vector.tensor_scalar(dp1[:], delta[:], 1.0, None, A.add)
            dp1e = f32t("dp1e", pool=ppool)
            nc.vector.tensor_scalar(dp1e[:], dp1[:], 1e-10, None, A.add)
            a_ = f32t("a_", pool=ppool)          # a = 1/(bb+1e-10)
            nc.vector.reciprocal(a_[:], dp1e[:])
            de = f32t("de", pool=ppool)
            nc.vector.tensor_scalar(de[:], delta[:], 1e-10, None, A.add)
            rd = f32t("rd", pool=ppool)          # 1/(d+1e-10)
            nc.vector.reciprocal(rd[:], de[:])

            dmr = f32t("dmr", pool=ppool)        # d - 1/d
            nc.vector.tensor_tensor(dmr[:], delta[:], rd[:], A.subtract)
            nhd = f32t("nhd", pool=ppool)        # -d/2
            nc.vector.tensor_scalar(nhd[:], hd[:], -1.0, None, A.mult)
            c_ = f32t("c_", pool=ppool)          # c = -(d/2)(d - 1/d)
            nc.vector.tensor_tensor(c_[:], dmr[:], nhd[:], A.mult)

            Dl = f32t("Dl", pool=ppool)          # D = bb - a
            nc.vector.tensor_tensor(Dl[:], dp1[:], a_[:], A.subtract)
            abb = f32t("abb", pool=ppool)        # a + bb
            nc.vector.tensor_tensor(abb[:], dp1[:], a_[:], A.add)
            al = f32t("al", pool=ppool)          # alpha = D*(a+bb)
            nc.vector.tensor_tensor(al[:], Dl[:], abb[:], A.mult)

            t2 = f32t("t2", pool=ppool)          # D + a + bb
            nc.vector.tensor_tensor(t2[:], Dl[:], abb[:], A.add)
            t3 = f32t("t3", pool=ppool)          # c*(D+a+bb)
            nc.vector.tensor_tensor(t3[:], t2[:], c_[:], A.mult)
            t4 = f32t("t4", pool=ppool)          # alpha*hd
            nc.vector.tensor_tensor(t4[:], al[:], hd[:], A.mult)
            t5 = f32t("t5", pool=ppool)          # 2*alpha*hd
            nc.vector.tensor_scalar(t5[:], t4[:], 2.0, None, A.mult)
            bp = f32t("bp", pool=ppool)          # beta
            nc.vector.tensor_tensor(bp[:], t5[:], t3[:], A.add)

            g1 = f32t("g1", pool=ppool)
            nc.vector.tensor_tensor(g1[:], hd[:], Dl[:], A.mult)
            g1c = f32t("g1c", pool=ppool)
            nc.vector.tensor_tensor(g1c[:], g1[:], c_[:], A.add)
            g2 = f32t("g2", pool=ppool)
            nc.vector.tensor_tensor(g2[:], hd[:], abb[:], A.mult)
            g2c = f32t("g2c", pool=ppool)
            nc.vector.tensor_tensor(g2c[:], g2[:], c_[:], A.add)
            gp = f32t("gp", pool=ppool)          # gamma
            nc.vector.tensor_tensor(gp[:], g1c[:], g2c[:], A.mult)

            # base-term planes (rhs of base matmuls), f32
            A2 = f32t("A2")                      # a^2
            nc.vector.tensor_tensor(A2[:], a_[:], a_[:], A.mult)
            acen = f32t("acen", pool=ppool)
            nc.vector.tensor_tensor(acen[:], a_[:], cen[:], A.mult)
            A2C2 = f32t("A2C2")                  # (a*cen)^2
            nc.vector.tensor_tensor(A2C2[:], acen[:], acen[:], A.mult)
            t6 = f32t("t6", pool=ppool)
            nc.vector.tensor_tensor(t6[:], A2[:], cen[:], A.mult)
            M2AC = f32t("M2AC")                  # -2*a^2*cen
            nc.vector.tensor_scalar(M2AC[:], t6[:], -2.0, None, A.mult)

            # 16-bit copies of per-label matmul coefficients. Planes are
            # hd-rescaled: m' = m/hd, m2' = (m/hd)^2 (bf16), s unchanged, so
            # the rhs columns absorb the compensation:
            #   sum alpha*m^2 = sum (alpha*hd^2) * m2'
            #   sum beta *m   = sum (beta*hd)    * m'
            ah1 = f32t("ah1", pool=ppool)        # alpha*hd
            nc.vector.tensor_tensor(ah1[:], al[:], hd[:], A.mult)
            ah = f32t("ah", pool=ppool)          # alpha*hd^2
            nc.vector.tensor_tensor(ah[:], ah1[:], hd[:], A.mult)
            bh = f32t("bh", pool=ppool)          # beta*hd
            nc.vector.tensor_tensor(bh[:], bp[:], hd[:], A.mult)
            al16 = cpool.tile([H, LPC], BF16, tag="al16")
            nc.vector.tensor_copy(al16[:], ah[:])
            bp16 = cpool.tile([H, LPC], F16, tag="bp16")
            nc.vector.tensor_copy(bp16[:], bh[:])
            gp16 = cpool.tile([H, LPC], F16, tag="gp16")
            nc.vector.tensor_copy(gp16[:], gp[:])

            ppool_cm.__exit__(None, None, None)
            l1pool = ctx.enter_context(tc.tile_pool(name="l1", bufs=2))
            mpool = ctx.enter_context(tc.tile_pool(name="m", bufs=2))
            spool = ctx.enter_context(tc.tile_pool(name="s", bufs=2))
            m2pool = ctx.enter_context(tc.tile_pool(name="m2", bufs=2))
            opool = ctx.enter_context(tc.tile_pool(name="outs", bufs=2))
            x2T = cpool.tile([H, B], F32, tag="x2T")   # x^2
            nc.vector.tensor_tensor(x2T[:], xT[:], xT[:], A.mult)
            ones = cpool.tile([H, 128], F32, tag="ones")
            nc.gpsimd.memset(ones[:], 1.0)
            x16 = cpool.tile([H, B], F16, tag="x16")   # fp16 x for DVE l1 path
            nc.vector.tensor_copy(x16[:], xT[:])
            mask = cpool.tile([H, 1], mybir.dt.uint16, tag="mask")
            nc.gpsimd.memset(mask[:], 0x7FFF)          # fp16 sign-bit clear

            # ---- base matmuls into PSUM: dist2 base term ----
            # (repeat>1 re-runs the compute body in a HW loop for timing;
            # each iteration recomputes the same outputs)
            tiles = dict(xT=xT, x2T=x2T, ones=ones, hd=hd, invhd=invhd,
                         cod=cod, ncod=ncod, x16=x16, mask=mask, A2=A2,
                         M2AC=M2AC, A2C2=A2C2, al16=al16, bp16=bp16,
                         gp16=gp16)
            if repeat > 1:
                with tc.For_i(0, repeat, 1):
                    _run_body(nc, tc, l1pool, mpool, spool, m2pool,
                              pspool, opool, tiles, out_d, ablate)
            else:
                _run_body(nc, tc, l1pool, mpool, spool, m2pool, pspool,
                          opool, tiles, out_d, ablate)

    nc.compile()
    return nc


def _run_body(nc, tc, l1pool, mpool, spool, m2pool, pspool, opool,
              tiles, out_d, ablate=frozenset()):
            U16 = mybir.dt.uint16
            xT, x2T, ones = tiles["xT"], tiles["x2T"], tiles["ones"]
            hd, invhd = tiles["hd"], tiles["invhd"]
            cod, ncod = tiles["cod"], tiles["ncod"]
            x16, mask = tiles["x16"], tiles["mask"]
            A2, M2AC, A2C2 = tiles["A2"], tiles["M2AC"], tiles["A2C2"]
            al16, bp16, gp16 = tiles["al16"], tiles["bp16"], tiles["gp16"]
            psts = []
            for cch in range(NBCH):
                pst = pspool.tile([128, LPC], F32, tag=f"ps{cch}")
                psts.append(pst)
                sl = bass.ts(cch, 128)
                nc.tensor.matmul(pst[:], x2T[:, sl], A2[:],
                                 start=True, stop=False, skip_group_check=True)
                nc.tensor.matmul(pst[:], xT[:, sl], M2AC[:],
                                 start=False, stop=False, skip_group_check=True)
                nc.tensor.matmul(pst[:], ones[:], A2C2[:],
                                 start=False, stop=False, skip_group_check=True)

            # ---- per-label planes + PE reductions ----
            # Rescaled space: l1' = |x - cen| / hd, so the inside/outside
            # threshold is the immediate 1.0 for every label — the m'/s
            # tensor_scalar ops then use immediate scalars (DVE 4x mode) and
            # batch G labels per instruction. l1' is produced on ACT (Abs
            # with per-partition scale+bias) for 3 of every G=4 labels and
            # on DVE (fp16 mult-sub + sign-bit and) for the 4th.
            for g in range(LPC // GRP):
                l0 = g * GRP
                l1g = l1pool.tile([H, GRP * B], F16, tag="l1g")
                # trailing DVE-path labels (6 ACT : 2 DVE balances the
                # ScalarE Abs cost against the DVE plane ops)
                ndve = 2
                nact = GRP - ndve
                tg = l1pool.tile([H, 3 * B], F16, tag="tg")
                for j in range(GRP):
                    l = l0 + j
                    lsl = slice(l, l + 1)
                    gsl = slice(j * B, (j + 1) * B)
                    if ("noact" not in ablate) and j < nact:
                        nc.scalar.activation(l1g[:, gsl], xT[:], ACT.Abs,
                                             bias=ncod[:, lsl],
                                             scale=invhd[:, lsl])
                    else:
                        tsl = slice((j - nact) * B, (j - nact + 1) * B)
                        nc.vector.tensor_scalar(tg[:, tsl], x16[:],
                                                invhd[:, lsl], cod[:, lsl],
                                                A.mult, A.subtract)
                # one grouped sign-bit and for all DVE-path labels
                nc.vector.tensor_scalar(
                    l1g.bitcast(U16)[:, nact * B:GRP * B],
                    tg.bitcast(U16)[:, 0:ndve * B], 0x7FFF, None,
                    A.bitwise_and)
                m = mpool.tile([H, GRP * B], F16, tag="m")
                nc.vector.tensor_scalar(m[:], l1g[:], 1.0, 0.0,
                                        A.subtract, A.max)
                s = spool.tile([H, GRP * B], F16, tag="s")
                nc.vector.tensor_scalar(s[:], l1g[:], 1.0, None, A.is_gt)
                if "m2" in ablate:
                    m2 = m
                else:
                    m2 = m2pool.tile([H, GRP * B], BF16, tag="m2")
                    nc.vector.tensor_tensor(m2[:], m[:], m[:], A.mult)

                if "pe" in ablate:
                    continue
                for j in range(GRP):
                    l = l0 + j
                    lsl = slice(l, l + 1)
                    last = l == LPC - 1
                    for cch in range(NBCH):
                        sl = slice(j * B + cch * 128, j * B + (cch + 1) * 128)
                        pcol = psts[cch][:, lsl]
                        nc.tensor.matmul(pcol, m2[:, sl], al16[:, lsl],
                                         start=False, stop=False,
                                         skip_group_check=True)
                        nc.tensor.matmul(pcol, m[:, sl], bp16[:, lsl],
                                         start=False, stop=False,
                                         skip_group_check=True)
                        nc.tensor.matmul(pcol, s[:, sl], gp16[:, lsl],
                                         start=False, stop=last,
                                         skip_group_check=True)

            # ---- finalize: out = -sqrt(psum) ----
            for cch in range(NBCH):
                sq = opool.tile([128, LPC], F32, tag="sq")
                nc.scalar.activation(sq[:], psts[cch][:], ACT.Sqrt)
                o = opool.tile([128, LPC], F32, tag="o")
                nc.vector.tensor_scalar(o[:], sq[:], -1.0, None, A.mult)
                nc.sync.dma_start(out_d.ap()[bass.ts(cch, 128), :], o[:])


_NC_CACHE = None


def _get_nc():
    global _NC_CACHE
    if _NC_CACHE is None:
        _NC_CACHE = build_nc()
    return _NC_CACHE


def kernel(y: np.ndarray, x: np.ndarray) -> np.ndarray:
    y = np.asarray(y, dtype=np.float32)
    x = np.asarray(x, dtype=np.float32)
    assert y.shape == (L, 2 * H) and x.shape == (B, H)

    nc = _get_nc()
    xT = np.ascontiguousarray(x.T)                       # (H, B)
    in_maps = []
    for c in range(N_CORES):
        ys = y[c * LPC:(c + 1) * LPC]
        in_maps.append({
            "xT": xT,
            "mnT": np.ascontiguousarray(ys[:, :H].T),    # (H, LPC)
            "rawT": np.ascontiguousarray(ys[:, H:].T),   # (H, LPC)
        })
    res = bass_utils.run_bass_kernel_spmd(nc, in_maps,
                                          core_ids=list(range(N_CORES)))
    out = np.concatenate([res.results[c]["out"] for c in range(N_CORES)],
                         axis=1)
    return np.ascontiguousarray(out.astype(np.float32))



# revision 5
# speedup vs baseline: 2.4865x; 2.4865x over previous
"""BoxE scorer kernel for Trainium2 (8 NeuronCores, label-sharded).

Computes out[b,l] = -|| per_dim(x[b], box[l]) ||_2 for
  y: (2048, 256) f32   (per-label box params: mn = y[:, :128], delta = softplus(y[:, 128:]))
  x: (1024, 128) f32
  out: (1024, 2048) f32

Math: with d = softplus(raw), hd = d/2, bb = d+1, a = 1/(bb+1e-10),
cen = mn + hd, c = -hd*(d - 1/(d+1e-10)), l1 = |x - cen|,
rescaled l1' = l1/hd, m' = relu(l1' - 1), s = [l1' > 1]:
  dist2 = base + sum_h (alpha*hd^2)*m'^2 + (beta*hd)*m' + gamma*s
  alpha = bb^2 - a^2;  beta = 2*hd*alpha + 2*bb*c
  gamma = (bb*hd + c)^2 - (a*hd)^2
  base  = sum_h a^2*x^2 - 2*a^2*cen*x + (a*cen)^2   (3 plain matmuls)

Engine split (per core: 256 labels over 32 groups of 8):
 - l1' planes: DVE for most groups (per-label dual-op mult-subtract at 4x
   + one grouped sign-clear AND per group), ACT Abs for ACT_L1_GROUPS
   groups (per-label scale/bias) to balance engine load.
 - m', s: grouped DVE tensor_scalar (immediate thresholds, 4x fp16).
 - m'^2: one grouped ACT Square instruction per group (bf16) — this
   replaces the baseline's dominant DVE tensor_tensor.
 - PE: base matmuls + 3 per-label weighted partition-reductions into
   PSUM; finalize -sqrt on ACT/DVE.
"""

import os
from contextlib import ExitStack

import numpy as np

import concourse.bass as bass
import concourse.tile as tile
from concourse import bacc, mybir
from concourse import bass_utils

F32 = mybir.dt.float32
F16 = mybir.dt.float16
BF16 = mybir.dt.bfloat16
U16 = mybir.dt.uint16
A = mybir.AluOpType
ACT = mybir.ActivationFunctionType

B = 1024      # batch
H = 128       # hidden
L = 2048      # num labels
N_CORES = 8
LPC = L // N_CORES   # labels per core
NBCH = B // 128      # batch chunks of 128
GRP = 8              # labels per grouped block
NGRP = LPC // GRP    # 32 groups
ACT_L1_GROUPS = 4    # groups whose l1' comes from ACT Abs (rest: DVE)


def build_nc(repeat: int = 1, ablate: frozenset = frozenset()):
    nc = bacc.Bacc("TRN2", target_bir_lowering=False, debug=False,
                   num_devices=N_CORES)
    xT_d = nc.dram_tensor("xT", (H, B), F32, kind="ExternalInput")
    mnT_d = nc.dram_tensor("mnT", (H, LPC), F32, kind="ExternalInput")
    rawT_d = nc.dram_tensor("rawT", (H, LPC), F32, kind="ExternalInput")
    out_d = nc.dram_tensor("out", (B, LPC), F32, kind="ExternalOutput")

    with tile.TileContext(nc) as tc:
        with ExitStack() as ctx:
            cpool = ctx.enter_context(tc.tile_pool(name="consts", bufs=1))
            pspool = ctx.enter_context(
                tc.tile_pool(name="psum", bufs=1, space=bass.MemorySpace.PSUM))

            # ---- load inputs ----
            ppool_cm = tc.tile_pool(name="pre", bufs=1)
            ppool = ppool_cm.__enter__()
            xT = cpool.tile([H, B], F32, tag="xT")
            nc.sync.dma_start(xT[:], xT_d.ap())
            mnT = ppool.tile([H, LPC], F32, tag="mnT")
            nc.sync.dma_start(mnT[:], mnT_d.ap())
            rawT = ppool.tile([H, LPC], F32, tag="rawT")
            nc.sync.dma_start(rawT[:], rawT_d.ap())

            def f32t(tag, shape=(H, LPC), pool=None):
                return (pool or cpool).tile(list(shape), F32, tag=tag,
                                            name=tag)

            # ---- per-label coefficient precompute (all [H, LPC] f32) ----
            # delta = softplus(raw) = ln(1 + exp(raw))
            e = f32t("e", pool=ppool)
            nc.scalar.activation(e[:], rawT[:], ACT.Exp)
            e1 = f32t("e1", pool=ppool)
            nc.vector.tensor_scalar(e1[:], e[:], 1.0, None, A.add)
            delta = f32t("delta", pool=ppool)
            nc.scalar.activation(delta[:], e1[:], ACT.Ln)

            bb = f32t("bb", pool=ppool)          # d + 1
            nc.vector.tensor_scalar(bb[:], delta[:], 1.0, None, A.add)
            bbe = f32t("bbe", pool=ppool)
            nc.vector.tensor_scalar(bbe[:], bb[:], 1e-10, None, A.add)
            a_ = f32t("a_", pool=ppool)          # 1/(bb+1e-10)
            nc.vector.reciprocal(a_[:], bbe[:])
            hd = f32t("hd", pool=ppool)          # d/2
            nc.vector.tensor_scalar(hd[:], delta[:], 0.5, None, A.mult)
            cen = f32t("cen", pool=ppool)        # mn + d/2
            nc.vector.tensor_tensor(cen[:], mnT[:], hd[:], A.add)
            invhd = f32t("invhd")                # 1/hd (kept: TS/ACT scale)
            nc.vector.reciprocal(invhd[:], hd[:])
            cod = f32t("cod")                    # cen/hd (kept: TS scalar)
            nc.vector.tensor_tensor(cod[:], cen[:], invhd[:], A.mult)
            ncod = f32t("ncod")                  # -cen/hd (kept: ACT bias)
            nc.vector.tensor_scalar(ncod[:], cod[:], -1.0, None, A.mult)

            de = f32t("de", pool=ppool)
            nc.vector.tensor_scalar(de[:], delta[:], 1e-10, None, A.add)
            rd = f32t("rd", pool=ppool)          # 1/(d+1e-10)
            nc.vector.reciprocal(rd[:], de[:])
            dmr = f32t("dmr", pool=ppool)        # d - 1/d
            nc.vector.tensor_tensor(dmr[:], delta[:], rd[:], A.subtract)
            nhd = f32t("nhd", pool=ppool)        # -d/2
            nc.vector.tensor_scalar(nhd[:], hd[:], -1.0, None, A.mult)
            c_ = f32t("c_", pool=ppool)          # c = -(d/2)(d - 1/d)
            nc.vector.tensor_tensor(c_[:], dmr[:], nhd[:], A.mult)

            u1 = f32t("u1", pool=ppool)          # bb - a
            nc.vector.tensor_tensor(u1[:], bb[:], a_[:], A.subtract)
            u2 = f32t("u2", pool=ppool)          # bb + a
            nc.vector.tensor_tensor(u2[:], bb[:], a_[:], A.add)
            alpha = f32t("alpha", pool=ppool)    # bb^2 - a^2
            nc.vector.tensor_tensor(alpha[:], u1[:], u2[:], A.mult)

            b1 = f32t("b1", pool=ppool)          # hd*alpha
            nc.vector.tensor_tensor(b1[:], hd[:], alpha[:], A.mult)
            b2 = f32t("b2", pool=ppool)          # bb*c
            nc.vector.tensor_tensor(b2[:], bb[:], c_[:], A.mult)
            b3 = f32t("b3", pool=ppool)
            nc.vector.tensor_tensor(b3[:], b1[:], b2[:], A.add)
            beta = f32t("beta", pool=ppool)      # 2(hd*alpha + bb*c)
            nc.vector.tensor_scalar(beta[:], b3[:], 2.0, None, A.mult)

            g1 = f32t("g1", pool=ppool)          # bb*hd
            nc.vector.tensor_tensor(g1[:], bb[:], hd[:], A.mult)
            g1c = f32t("g1c", pool=ppool)        # bb*hd + c
            nc.vector.tensor_tensor(g1c[:], g1[:], c_[:], A.add)
            g2 = f32t("g2", pool=ppool)          # a*hd
            nc.vector.tensor_tensor(g2[:], a_[:], hd[:], A.mult)
            gm = f32t("gm", pool=ppool)
            nc.vector.tensor_tensor(gm[:], g1c[:], g2[:], A.subtract)
            gpl = f32t("gpl", pool=ppool)
            nc.vector.tensor_tensor(gpl[:], g1c[:], g2[:], A.add)
            gamma = f32t("gamma", pool=ppool)    # (bb*hd+c)^2 - (a*hd)^2
            nc.vector.tensor_tensor(gamma[:], gm[:], gpl[:], A.mult)

            # rescaled 16-bit weight columns: alpha*hd^2 (bf16, pairs with
            # bf16 m'^2), beta*hd and gamma (fp16)
            ah1 = f32t("ah1", pool=ppool)        # alpha*hd
            nc.vector.tensor_tensor(ah1[:], alpha[:], hd[:], A.mult)
            ah2 = f32t("ah2", pool=ppool)        # alpha*hd^2
            nc.vector.tensor_tensor(ah2[:], ah1[:], hd[:], A.mult)
            bh = f32t("bh", pool=ppool)          # beta*hd
            nc.vector.tensor_tensor(bh[:], beta[:], hd[:], A.mult)
            w2c = cpool.tile([H, LPC], BF16, tag="w2c")
            nc.vector.tensor_copy(w2c[:], ah2[:])
            w1c = cpool.tile([H, LPC], F16, tag="w1c")
            nc.vector.tensor_copy(w1c[:], bh[:])
            w0c = cpool.tile([H, LPC], F16, tag="w0c")
            nc.vector.tensor_copy(w0c[:], gamma[:])

            # base-term planes (rhs of base matmuls), f32
            A2 = f32t("A2")                      # a^2
            nc.vector.tensor_tensor(A2[:], a_[:], a_[:], A.mult)
            acen = f32t("acen", pool=ppool)
            nc.vector.tensor_tensor(acen[:], a_[:], cen[:], A.mult)
            A2C2 = f32t("A2C2")                  # (a*cen)^2
            nc.vector.tensor_tensor(A2C2[:], acen[:], acen[:], A.mult)
            t6 = f32t("t6", pool=ppool)
            nc.vector.tensor_tensor(t6[:], A2[:], cen[:], A.mult)
            M2AC = f32t("M2AC")                  # -2*a^2*cen
            nc.vector.tensor_scalar(M2AC[:], t6[:], -2.0, None, A.mult)

            ppool_cm.__exit__(None, None, None)
            l1pool = ctx.enter_context(tc.tile_pool(name="l1", bufs=2))
            mpool = ctx.enter_context(tc.tile_pool(name="m", bufs=2))
            spool = ctx.enter_context(tc.tile_pool(name="s", bufs=2))
            m2pool = ctx.enter_context(tc.tile_pool(name="m2", bufs=2))
            opool = ctx.enter_context(tc.tile_pool(name="outs", bufs=2))
            x2T = cpool.tile([H, B], F32, tag="x2T")   # x^2
            nc.vector.tensor_tensor(x2T[:], xT[:], xT[:], A.mult)
            ones = cpool.tile([H, 128], F32, tag="ones")
            nc.gpsimd.memset(ones[:], 1.0)
            x16 = cpool.tile([H, B], F16, tag="x16")   # fp16 x for DVE planes
            nc.vector.tensor_copy(x16[:], xT[:])

            tiles = dict(xT=xT, x2T=x2T, ones=ones, invhd=invhd, cod=cod,
                         ncod=ncod, x16=x16, A2=A2, M2AC=M2AC, A2C2=A2C2,
                         w2c=w2c, w1c=w1c, w0c=w0c)
            if repeat > 1:
                with tc.For_i(0, repeat, 1):
                    _run_body(nc, tc, l1pool, mpool, spool, m2pool,
                              pspool, opool, tiles, out_d, ablate)
            else:
                _run_body(nc, tc, l1pool, mpool, spool, m2pool, pspool,
                          opool, tiles, out_d, ablate)

    nc.compile()
    return nc


def _run_body(nc, tc, l1pool, mpool, spool, m2pool, pspool, opool,
              tiles, out_d, ablate=frozenset()):
            xT, x2T, ones = tiles["xT"], tiles["x2T"], tiles["ones"]
            invhd, cod, ncod = tiles["invhd"], tiles["cod"], tiles["ncod"]
            x16 = tiles["x16"]
            A2, M2AC, A2C2 = tiles["A2"], tiles["M2AC"], tiles["A2C2"]
            w2c, w1c, w0c = tiles["w2c"], tiles["w1c"], tiles["w0c"]

            # ---- base matmuls into PSUM: dist2 base term ----
            psts = []
            for cch in range(NBCH):
                pst = pspool.tile([128, LPC], F32, tag=f"ps{cch}")
                psts.append(pst)
                sl = bass.ts(cch, 128)
                nc.tensor.matmul(pst[:], x2T[:, sl], A2[:],
                                 start=True, stop=False, skip_group_check=True)
                nc.tensor.matmul(pst[:], xT[:, sl], M2AC[:],
                                 start=False, stop=False, skip_group_check=True)
                nc.tensor.matmul(pst[:], ones[:], A2C2[:],
                                 start=False, stop=False, skip_group_check=True)

            # ---- per-label planes + PE reductions ----
            for g in range(NGRP):
                l0 = g * GRP
                l1g = l1pool.tile([H, GRP * B], F16, tag="l1g")
                if ("noact" in ablate) or g >= ACT_L1_GROUPS:
                    # DVE path: u = x/hd - cen/hd per label, then one
                    # grouped sign-clear AND -> l1' planes
                    for j in range(GRP):
                        l = l0 + j
                        lsl = slice(l, l + 1)
                        gsl = slice(j * B, (j + 1) * B)
                        nc.vector.tensor_scalar(l1g[:, gsl], x16[:],
                                                invhd[:, lsl], cod[:, lsl],
                                                A.mult, A.subtract)
                    nc.vector.tensor_scalar(
                        l1g.bitcast(U16)[:], l1g.bitcast(U16)[:],
                        0x7FFF, None, A.bitwise_and)
                else:
                    # ACT path: l1' = Abs(x*invhd - cod) per label
                    for j in range(GRP):
                        l = l0 + j
                        lsl = slice(l, l + 1)
                        gsl = slice(j * B, (j + 1) * B)
                        nc.scalar.activation(l1g[:, gsl], xT[:], ACT.Abs,
                                             bias=ncod[:, lsl],
                                             scale=invhd[:, lsl])
                m = mpool.tile([H, GRP * B], F16, tag="m")
                nc.vector.tensor_scalar(m[:], l1g[:], 1.0, 0.0,
                                        A.subtract, A.max)
                s = spool.tile([H, GRP * B], F16, tag="s")
                nc.vector.tensor_scalar(s[:], l1g[:], 1.0, None, A.is_gt)
                if "m2" in ablate:
                    m2 = m
                else:
                    m2 = m2pool.tile([H, GRP * B], BF16, tag="m2")
                    nc.scalar.square(m2[:], m[:])   # grouped ACT Square

                if "pe" in ablate:
                    continue
                for j in range(GRP):
                    l = l0 + j
                    lsl = slice(l, l + 1)
                    last = l == LPC - 1
                    for cch in range(NBCH):
                        sl = slice(j * B + cch * 128, j * B + (cch + 1) * 128)
                        pcol = psts[cch][:, lsl]
                        nc.tensor.matmul(pcol, m2[:, sl], w2c[:, lsl],
                                         start=False, stop=False,
                                         skip_group_check=True)
                        nc.tensor.matmul(pcol, m[:, sl], w1c[:, lsl],
                                         start=False, stop=False,
                                         skip_group_check=True)
                        nc.tensor.matmul(pcol, s[:, sl], w0c[:, lsl],
                                         start=False, stop=last,
                                         skip_group_check=True)

            # ---- finalize: out = -sqrt(psum) ----
            for cch in range(NBCH):
                sq = opool.tile([128, LPC], F32, tag="sq")
                nc.scalar.sqrt(sq[:], psts[cch][:])
                o = opool.tile([128, LPC], F32, tag="o")
                nc.vector.tensor_scalar(o[:], sq[:], -1.0, None, A.mult)
                nc.sync.dma_start(out_d.ap()[bass.ts(cch, 128), :], o[:])


_NC_CACHE = None


def _get_nc():
    global _NC_CACHE
    if _NC_CACHE is None:
        _NC_CACHE = build_nc()
    return _NC_CACHE


def kernel(y: np.ndarray, x: np.ndarray) -> np.ndarray:
    y = np.asarray(y, dtype=np.float32)
    x = np.asarray(x, dtype=np.float32)
    assert y.shape == (L, 2 * H) and x.shape == (B, H)

    nc = _get_nc()
    xT = np.ascontiguousarray(x.T)                       # (H, B)
    in_maps = []
    for c in range(N_CORES):
        ys = y[c * LPC:(c + 1) * LPC]
        in_maps.append({
            "xT": xT,
            "mnT": np.ascontiguousarray(ys[:, :H].T),    # (H, LPC)
            "rawT": np.ascontiguousarray(ys[:, H:].T),   # (H, LPC)
        })
    res = bass_utils.run_bass_kernel_spmd(nc, in_maps,
                                          core_ids=list(range(N_CORES)))
    out = np.concatenate([res.results[c]["out"] for c in range(N_CORES)],
                         axis=1)
    return np.ascontiguousarray(out.astype(np.float32))


# revision 6
# speedup vs baseline: 2.5133x; 1.0108x over previous
"""BoxE scorer kernel for Trainium2 (8 NeuronCores, label-sharded).

Computes out[b,l] = -|| per_dim(x[b], box[l]) ||_2 for
  y: (2048, 256) f32   (per-label box params: mn = y[:, :128], delta = softplus(y[:, 128:]))
  x: (1024, 128) f32
  out: (1024, 2048) f32

Math: with d = softplus(raw), hd = d/2, bb = d+1, a = 1/(bb+1e-10),
cen = mn + hd, c = -hd*(d - 1/(d+1e-10)), l1 = |x - cen|,
rescaled l1' = l1/hd, m' = relu(l1' - 1), s = [l1' > 1]:
  dist2 = base + sum_h (alpha*hd^2)*m'^2 + (beta*hd)*m' + gamma*s
  alpha = bb^2 - a^2;  beta = 2*hd*alpha + 2*bb*c
  gamma = (bb*hd + c)^2 - (a*hd)^2
  base  = sum_h a^2*x^2 - 2*a^2*cen*x + (a*cen)^2   (3 plain matmuls)

Engine split (per core: 256 labels over 32 groups of 8):
 - l1' planes: DVE for most groups (per-label dual-op mult-subtract at 4x
   + one grouped sign-clear AND per group), ACT Abs for ACT_L1_GROUPS
   groups (per-label scale/bias) to balance engine load.
 - m', s: grouped DVE tensor_scalar (immediate thresholds, 4x fp16).
 - m'^2: one grouped ACT Square instruction per group (bf16) — this
   replaces the baseline's dominant DVE tensor_tensor.
 - PE: base matmuls + 3 per-label weighted partition-reductions into
   PSUM; finalize -sqrt on ACT/DVE.
"""

import os
from contextlib import ExitStack

import numpy as np

import concourse.bass as bass
import concourse.tile as tile
from concourse import bacc, mybir
from concourse import bass_utils

F32 = mybir.dt.float32
F16 = mybir.dt.float16
BF16 = mybir.dt.bfloat16
U16 = mybir.dt.uint16
A = mybir.AluOpType
ACT = mybir.ActivationFunctionType

B = 1024      # batch
H = 128       # hidden
L = 2048      # num labels
N_CORES = 8
LPC = L // N_CORES   # labels per core
NBCH = B // 128      # batch chunks of 128
GRP = 8              # labels per grouped block
NGRP = LPC // GRP    # 32 groups
ACT_L1_GROUPS = 4    # groups whose l1' comes from ACT Abs (rest: DVE)


def build_nc(repeat: int = 1, ablate: frozenset = frozenset()):
    nc = bacc.Bacc("TRN2", target_bir_lowering=False, debug=False,
                   num_devices=N_CORES)
    xT_d = nc.dram_tensor("xT", (H, B), F32, kind="ExternalInput")
    mnT_d = nc.dram_tensor("mnT", (H, LPC), F32, kind="ExternalInput")
    rawT_d = nc.dram_tensor("rawT", (H, LPC), F32, kind="ExternalInput")
    out_d = nc.dram_tensor("out", (B, LPC), F32, kind="ExternalOutput")

    with tile.TileContext(nc) as tc:
        with ExitStack() as ctx:
            cpool = ctx.enter_context(tc.tile_pool(name="consts", bufs=1))
            pspool = ctx.enter_context(
                tc.tile_pool(name="psum", bufs=1, space=bass.MemorySpace.PSUM))

            # ---- load inputs ----
            ppool_cm = tc.tile_pool(name="pre", bufs=1)
            ppool = ppool_cm.__enter__()
            xT = cpool.tile([H, B], F32, tag="xT")
            nc.sync.dma_start(xT[:], xT_d.ap())
            mnT = ppool.tile([H, LPC], F32, tag="mnT")
            nc.sync.dma_start(mnT[:], mnT_d.ap())
            rawT = ppool.tile([H, LPC], F32, tag="rawT")
            nc.sync.dma_start(rawT[:], rawT_d.ap())

            def f32t(tag, shape=(H, LPC), pool=None):
                return (pool or cpool).tile(list(shape), F32, tag=tag,
                                            name=tag)

            # ---- per-label coefficient precompute (all [H, LPC] f32) ----
            # delta = softplus(raw) = ln(1 + exp(raw))
            e = f32t("e", pool=ppool)
            nc.scalar.activation(e[:], rawT[:], ACT.Exp)
            e1 = f32t("e1", pool=ppool)
            nc.vector.tensor_scalar(e1[:], e[:], 1.0, None, A.add)
            delta = f32t("delta", pool=ppool)
            nc.scalar.activation(delta[:], e1[:], ACT.Ln)

            bb = f32t("bb", pool=ppool)          # d + 1
            nc.vector.tensor_scalar(bb[:], delta[:], 1.0, None, A.add)
            bbe = f32t("bbe", pool=ppool)
            nc.vector.tensor_scalar(bbe[:], bb[:], 1e-10, None, A.add)
            a_ = f32t("a_", pool=ppool)          # 1/(bb+1e-10)
            nc.vector.reciprocal(a_[:], bbe[:])
            hd = f32t("hd", pool=ppool)          # d/2
            nc.vector.tensor_scalar(hd[:], delta[:], 0.5, None, A.mult)
            cen = f32t("cen", pool=ppool)        # mn + d/2
            nc.vector.tensor_tensor(cen[:], mnT[:], hd[:], A.add)
            invhd = f32t("invhd")                # 1/hd (kept: TS/ACT scale)
            nc.vector.reciprocal(invhd[:], hd[:])
            cod = f32t("cod")                    # cen/hd (kept: TS scalar)
            nc.vector.tensor_tensor(cod[:], cen[:], invhd[:], A.mult)
            ncod = f32t("ncod")                  # -cen/hd (kept: ACT bias)
            nc.vector.tensor_scalar(ncod[:], cod[:], -1.0, None, A.mult)

            de = f32t("de", pool=ppool)
            nc.vector.tensor_scalar(de[:], delta[:], 1e-10, None, A.add)
            rd = f32t("rd", pool=ppool)          # 1/(d+1e-10)
            nc.vector.reciprocal(rd[:], de[:])
            dmr = f32t("dmr", pool=ppool)        # d - 1/d
            nc.vector.tensor_tensor(dmr[:], delta[:], rd[:], A.subtract)
            nhd = f32t("nhd", pool=ppool)        # -d/2
            nc.vector.tensor_scalar(nhd[:], hd[:], -1.0, None, A.mult)
            c_ = f32t("c_", pool=ppool)          # c = -(d/2)(d - 1/d)
            nc.vector.tensor_tensor(c_[:], dmr[:], nhd[:], A.mult)

            u1 = f32t("u1", pool=ppool)          # bb - a
            nc.vector.tensor_tensor(u1[:], bb[:], a_[:], A.subtract)
            u2 = f32t("u2", pool=ppool)          # bb + a
            nc.vector.tensor_tensor(u2[:], bb[:], a_[:], A.add)
            alpha = f32t("alpha", pool=ppool)    # bb^2 - a^2
            nc.vector.tensor_tensor(alpha[:], u1[:], u2[:], A.mult)

            b1 = f32t("b1", pool=ppool)          # hd*alpha
            nc.vector.tensor_tensor(b1[:], hd[:], alpha[:], A.mult)
            b2 = f32t("b2", pool=ppool)          # bb*c
            nc.vector.tensor_tensor(b2[:], bb[:], c_[:], A.mult)
            b3 = f32t("b3", pool=ppool)
            nc.vector.tensor_tensor(b3[:], b1[:], b2[:], A.add)
            beta = f32t("beta", pool=ppool)      # 2(hd*alpha + bb*c)
            nc.vector.tensor_scalar(beta[:], b3[:], 2.0, None, A.mult)

            g1 = f32t("g1", pool=ppool)          # bb*hd
            nc.vector.tensor_tensor(g1[:], bb[:], hd[:], A.mult)
            g1c = f32t("g1c", pool=ppool)        # bb*hd + c
            nc.vector.tensor_tensor(g1c[:], g1[:], c_[:], A.add)
            g2 = f32t("g2", pool=ppool)          # a*hd
            nc.vector.tensor_tensor(g2[:], a_[:], hd[:], A.mult)
            gm = f32t("gm", pool=ppool)
            nc.vector.tensor_tensor(gm[:], g1c[:], g2[:], A.subtract)
            gpl = f32t("gpl", pool=ppool)
            nc.vector.tensor_tensor(gpl[:], g1c[:], g2[:], A.add)
            gamma = f32t("gamma", pool=ppool)    # (bb*hd+c)^2 - (a*hd)^2
            nc.vector.tensor_tensor(gamma[:], gm[:], gpl[:], A.mult)

            # rescaled 16-bit weight columns: alpha*hd^2 (bf16, pairs with
            # bf16 m'^2), beta*hd and gamma (fp16)
            ah1 = f32t("ah1", pool=ppool)        # alpha*hd
            nc.vector.tensor_tensor(ah1[:], alpha[:], hd[:], A.mult)
            ah2 = f32t("ah2", pool=ppool)        # alpha*hd^2
            nc.vector.tensor_tensor(ah2[:], ah1[:], hd[:], A.mult)
            bh = f32t("bh", pool=ppool)          # beta*hd
            nc.vector.tensor_tensor(bh[:], beta[:], hd[:], A.mult)
            w2c = cpool.tile([H, LPC], BF16, tag="w2c")
            nc.vector.tensor_copy(w2c[:], ah2[:])
            w1c = cpool.tile([H, LPC], F16, tag="w1c")
            nc.vector.tensor_copy(w1c[:], bh[:])
            w0c = cpool.tile([H, LPC], F16, tag="w0c")
            nc.vector.tensor_copy(w0c[:], gamma[:])

            # base-term planes (rhs of base matmuls), f32
            A2 = f32t("A2")                      # a^2
            nc.vector.tensor_tensor(A2[:], a_[:], a_[:], A.mult)
            acen = f32t("acen", pool=ppool)
            nc.vector.tensor_tensor(acen[:], a_[:], cen[:], A.mult)
            A2C2 = f32t("A2C2")                  # (a*cen)^2
            nc.vector.tensor_tensor(A2C2[:], acen[:], acen[:], A.mult)
            t6 = f32t("t6", pool=ppool)
            nc.vector.tensor_tensor(t6[:], A2[:], cen[:], A.mult)
            M2AC = f32t("M2AC")                  # -2*a^2*cen
            nc.vector.tensor_scalar(M2AC[:], t6[:], -2.0, None, A.mult)

            ppool_cm.__exit__(None, None, None)
            l1pool = ctx.enter_context(tc.tile_pool(name="l1", bufs=2))
            mpool = ctx.enter_context(tc.tile_pool(name="m", bufs=2))
            spool = ctx.enter_context(tc.tile_pool(name="s", bufs=2))
            m2pool = ctx.enter_context(tc.tile_pool(name="m2", bufs=2))
            opool = ctx.enter_context(tc.tile_pool(name="outs", bufs=2))
            x2T = cpool.tile([H, B], F32, tag="x2T")   # x^2
            nc.vector.tensor_tensor(x2T[:], xT[:], xT[:], A.mult)
            ones = cpool.tile([H, 128], F32, tag="ones")
            nc.gpsimd.memset(ones[:], 1.0)
            x16 = cpool.tile([H, B], F16, tag="x16")   # fp16 x for DVE planes
            nc.vector.tensor_copy(x16[:], xT[:])

            tiles = dict(xT=xT, x2T=x2T, ones=ones, invhd=invhd, cod=cod,
                         ncod=ncod, x16=x16, A2=A2, M2AC=M2AC, A2C2=A2C2,
                         w2c=w2c, w1c=w1c, w0c=w0c)
            if repeat > 1:
                with tc.For_i(0, repeat, 1):
                    _run_body(nc, tc, l1pool, mpool, spool, m2pool,
                              pspool, opool, tiles, out_d, ablate)
            else:
                _run_body(nc, tc, l1pool, mpool, spool, m2pool, pspool,
                          opool, tiles, out_d, ablate)

    nc.compile()
    return nc


def _run_body(nc, tc, l1pool, mpool, spool, m2pool, pspool, opool,
              tiles, out_d, ablate=frozenset()):
            xT, x2T, ones = tiles["xT"], tiles["x2T"], tiles["ones"]
            invhd, cod, ncod = tiles["invhd"], tiles["cod"], tiles["ncod"]
            x16 = tiles["x16"]
            A2, M2AC, A2C2 = tiles["A2"], tiles["M2AC"], tiles["A2C2"]
            w2c, w1c, w0c = tiles["w2c"], tiles["w1c"], tiles["w0c"]

            # ---- base matmuls into PSUM: dist2 base term ----
            psts = []
            for cch in range(NBCH):
                pst = pspool.tile([128, LPC], F32, tag=f"ps{cch}")
                psts.append(pst)
                sl = bass.ts(cch, 128)
                nc.tensor.matmul(pst[:], x2T[:, sl], A2[:],
                                 start=True, stop=False, skip_group_check=True)
                nc.tensor.matmul(pst[:], xT[:, sl], M2AC[:],
                                 start=False, stop=False, skip_group_check=True)
                nc.tensor.matmul(pst[:], ones[:], A2C2[:],
                                 start=False, stop=False, skip_group_check=True)

            # ---- per-label planes + PE reductions ----
            for g in range(NGRP):
                l0 = g * GRP
                l1g = l1pool.tile([H, GRP * B], F16, tag="l1g")
                if ("noact" in ablate) or g >= ACT_L1_GROUPS:
                    # DVE path: u = x/hd - cen/hd per label, then one
                    # grouped sign-clear AND -> l1' planes
                    tg = l1pool.tile([H, GRP * B], F16, tag="tg")
                    for j in range(GRP):
                        l = l0 + j
                        lsl = slice(l, l + 1)
                        gsl = slice(j * B, (j + 1) * B)
                        nc.vector.tensor_scalar(tg[:, gsl], x16[:],
                                                invhd[:, lsl], cod[:, lsl],
                                                A.mult, A.subtract)
                    nc.vector.tensor_scalar(
                        l1g.bitcast(U16)[:], tg.bitcast(U16)[:],
                        0x7FFF, None, A.bitwise_and)
                else:
                    # ACT path: l1' = Abs(x*invhd - cod) per label
                    for j in range(GRP):
                        l = l0 + j
                        lsl = slice(l, l + 1)
                        gsl = slice(j * B, (j + 1) * B)
                        nc.scalar.activation(l1g[:, gsl], xT[:], ACT.Abs,
                                             bias=ncod[:, lsl],
                                             scale=invhd[:, lsl])
                m = mpool.tile([H, GRP * B], F16, tag="m")
                nc.vector.tensor_scalar(m[:], l1g[:], 1.0, 0.0,
                                        A.subtract, A.max)
                s = spool.tile([H, GRP * B], F16, tag="s")
                nc.vector.tensor_scalar(s[:], l1g[:], 1.0, None, A.is_gt)
                if "m2" in ablate:
                    m2 = m
                else:
                    m2 = m2pool.tile([H, GRP * B], BF16, tag="m2")
                    nc.scalar.square(m2[:], m[:])   # grouped ACT Square

                if "pe" in ablate:
                    continue
                for j in range(GRP):
                    l = l0 + j
                    lsl = slice(l, l + 1)
                    last = l == LPC - 1
                    for cch in range(NBCH):
                        sl = slice(j * B + cch * 128, j * B + (cch + 1) * 128)
                        pcol = psts[cch][:, lsl]
                        nc.tensor.matmul(pcol, m2[:, sl], w2c[:, lsl],
                                         start=False, stop=False,
                                         skip_group_check=True)
                        nc.tensor.matmul(pcol, m[:, sl], w1c[:, lsl],
                                         start=False, stop=False,
                                         skip_group_check=True)
                        nc.tensor.matmul(pcol, s[:, sl], w0c[:, lsl],
                                         start=False, stop=last,
                                         skip_group_check=True)

            # ---- finalize: out = -sqrt(psum) ----
            for cch in range(NBCH):
                sq = opool.tile([128, LPC], F32, tag="sq")
                nc.scalar.sqrt(sq[:], psts[cch][:])
                o = opool.tile([128, LPC], F32, tag="o")
                nc.vector.tensor_scalar(o[:], sq[:], -1.0, None, A.mult)
                nc.sync.dma_start(out_d.ap()[bass.ts(cch, 128), :], o[:])


_NC_CACHE = None


def _get_nc():
    global _NC_CACHE
    if _NC_CACHE is None:
        _NC_CACHE = build_nc()
    return _NC_CACHE


def kernel(y: np.ndarray, x: np.ndarray) -> np.ndarray:
    y = np.asarray(y, dtype=np.float32)
    x = np.asarray(x, dtype=np.float32)
    assert y.shape == (L, 2 * H) and x.shape == (B, H)

    nc = _get_nc()
    xT = np.ascontiguousarray(x.T)                       # (H, B)
    in_maps = []
    for c in range(N_CORES):
        ys = y[c * LPC:(c + 1) * LPC]
        in_maps.append({
            "xT": xT,
            "mnT": np.ascontiguousarray(ys[:, :H].T),    # (H, LPC)
            "rawT": np.ascontiguousarray(ys[:, H:].T),   # (H, LPC)
        })
    res = bass_utils.run_bass_kernel_spmd(nc, in_maps,
                                          core_ids=list(range(N_CORES)))
    out = np.concatenate([res.results[c]["out"] for c in range(N_CORES)],
                         axis=1)
    return np.ascontiguousarray(out.astype(np.float32))


# revision 10
# speedup vs baseline: 2.9950x; 1.1916x over previous
"""BoxE scorer kernel for Trainium2 (8 NeuronCores, label-sharded).

Computes out[b,l] = -|| per_dim(x[b], box[l]) ||_2 for
  y: (2048, 256) f32   (per-label box params: mn = y[:, :128], delta = softplus(y[:, 128:]))
  x: (1024, 128) f32
  out: (1024, 2048) f32

Math: with d = softplus(raw), hd = d/2, bb = d+1, a = 1/(bb+1e-10),
cen = mn + hd, c = -hd*(d - 1/(d+1e-10)), l1 = |x - cen|,
rescaled l1' = l1/hd, m' = relu(l1' - 1), s = [l1' > 1]:
  dist2 = base + sum_h (alpha*hd^2)*m'^2 + (beta*hd)*m' + gamma*s
  alpha = bb^2 - a^2;  beta = 2*hd*alpha + 2*bb*c
  gamma = (bb*hd + c)^2 - (a*hd)^2
  base  = sum_h a^2*x^2 - 2*a^2*cen*x + (a*cen)^2   (3 plain matmuls)

Engine split (per core: 256 labels over 32 groups of 8):
 - l1' planes: DVE for most groups (per-label dual-op mult-subtract at 4x
   + one grouped sign-clear AND per group), ACT Abs for ACT_L1_GROUPS
   groups (per-label scale/bias) to balance engine load.
 - m', s: grouped DVE tensor_scalar (immediate thresholds, 4x fp16).
 - m'^2: one grouped ACT Square instruction per group (bf16) — this
   replaces the baseline's dominant DVE tensor_tensor.
 - PE: base matmuls + 3 per-label weighted partition-reductions into
   PSUM; finalize -sqrt on ACT/DVE.
"""

import os
from contextlib import ExitStack

import numpy as np

import concourse.bass as bass
import concourse.tile as tile
from concourse import bacc, mybir
from concourse import bass_utils

F32 = mybir.dt.float32
F16 = mybir.dt.float16
BF16 = mybir.dt.bfloat16
U16 = mybir.dt.uint16
A = mybir.AluOpType
ACT = mybir.ActivationFunctionType

B = 1024      # batch
H = 128       # hidden
L = 2048      # num labels
N_CORES = 8
LPC = L // N_CORES   # labels per core
NBCH = B // 128      # batch chunks of 128
GRP = 8              # labels per grouped block
NGRP = LPC // GRP    # 32 groups
ACT_L1_PER_GRP = 1   # labels per group whose l1' comes from ACT Abs


def build_nc(repeat: int = 1, ablate: frozenset = frozenset()):
    nc = bacc.Bacc("TRN2", target_bir_lowering=False, debug=False,
                   num_devices=N_CORES)
    xT_d = nc.dram_tensor("xT", (H, B), F32, kind="ExternalInput")
    mnT_d = nc.dram_tensor("mnT", (H, LPC), F32, kind="ExternalInput")
    rawT_d = nc.dram_tensor("rawT", (H, LPC), F32, kind="ExternalInput")
    out_d = nc.dram_tensor("out", (B, LPC), F32, kind="ExternalOutput")

    with tile.TileContext(nc) as tc:
        with ExitStack() as ctx:
            cpool = ctx.enter_context(tc.tile_pool(name="consts", bufs=1))
            pspool = ctx.enter_context(
                tc.tile_pool(name="psum", bufs=1, space=bass.MemorySpace.PSUM))

            # ---- load inputs ----
            ppool_cm = tc.tile_pool(name="pre", bufs=1)
            ppool = ppool_cm.__enter__()
            xT = cpool.tile([H, B], F32, tag="xT")
            nc.sync.dma_start(xT[:], xT_d.ap())
            mnT = ppool.tile([H, LPC], F32, tag="mnT")
            nc.sync.dma_start(mnT[:], mnT_d.ap())
            rawT = ppool.tile([H, LPC], F32, tag="rawT")
            nc.sync.dma_start(rawT[:], rawT_d.ap())

            def f32t(tag, shape=(H, LPC), pool=None):
                return (pool or cpool).tile(list(shape), F32, tag=tag,
                                            name=tag)

            # ---- per-label coefficient precompute (all [H, LPC] f32) ----
            # delta = softplus(raw) = ln(1 + exp(raw))
            e = f32t("e", pool=ppool)
            nc.scalar.activation(e[:], rawT[:], ACT.Exp)
            e1 = f32t("e1", pool=ppool)
            nc.vector.tensor_scalar(e1[:], e[:], 1.0, None, A.add)
            delta = f32t("delta", pool=ppool)
            nc.scalar.activation(delta[:], e1[:], ACT.Ln)

            bb = f32t("bb", pool=ppool)          # d + 1
            nc.vector.tensor_scalar(bb[:], delta[:], 1.0, None, A.add)
            bbe = f32t("bbe", pool=ppool)
            nc.vector.tensor_scalar(bbe[:], bb[:], 1e-10, None, A.add)
            a_ = f32t("a_", pool=ppool)          # 1/(bb+1e-10)
            nc.vector.reciprocal(a_[:], bbe[:])
            hd = f32t("hd", pool=ppool)          # d/2
            nc.vector.tensor_scalar(hd[:], delta[:], 0.5, None, A.mult)
            cen = f32t("cen", pool=ppool)        # mn + d/2
            nc.vector.tensor_tensor(cen[:], mnT[:], hd[:], A.add)
            invhd = f32t("invhd")                # 1/hd (kept: TS/ACT scale)
            nc.vector.reciprocal(invhd[:], hd[:])
            cod = f32t("cod")                    # cen/hd (kept: TS scalar)
            nc.vector.tensor_tensor(cod[:], cen[:], invhd[:], A.mult)
            ncod = f32t("ncod")                  # -cen/hd (kept: ACT bias)
            nc.vector.tensor_scalar(ncod[:], cod[:], -1.0, None, A.mult)

            de = f32t("de", pool=ppool)
            nc.vector.tensor_scalar(de[:], delta[:], 1e-10, None, A.add)
            rd = f32t("rd", pool=ppool)          # 1/(d+1e-10)
            nc.vector.reciprocal(rd[:], de[:])
            dmr = f32t("dmr", pool=ppool)        # d - 1/d
            nc.vector.tensor_tensor(dmr[:], delta[:], rd[:], A.subtract)
            nhd = f32t("nhd", pool=ppool)        # -d/2
            nc.vector.tensor_scalar(nhd[:], hd[:], -1.0, None, A.mult)
            c_ = f32t("c_", pool=ppool)          # c = -(d/2)(d - 1/d)
            nc.vector.tensor_tensor(c_[:], dmr[:], nhd[:], A.mult)

            u1 = f32t("u1", pool=ppool)          # bb - a
            nc.vector.tensor_tensor(u1[:], bb[:], a_[:], A.subtract)
            u2 = f32t("u2", pool=ppool)          # bb + a
            nc.vector.tensor_tensor(u2[:], bb[:], a_[:], A.add)
            alpha = f32t("alpha", pool=ppool)    # bb^2 - a^2
            nc.vector.tensor_tensor(alpha[:], u1[:], u2[:], A.mult)

            b1 = f32t("b1", pool=ppool)          # hd*alpha
            nc.vector.tensor_tensor(b1[:], hd[:], alpha[:], A.mult)
            b2 = f32t("b2", pool=ppool)          # bb*c
            nc.vector.tensor_tensor(b2[:], bb[:], c_[:], A.mult)
            b3 = f32t("b3", pool=ppool)
            nc.vector.tensor_tensor(b3[:], b1[:], b2[:], A.add)
            beta = f32t("beta", pool=ppool)      # 2(hd*alpha + bb*c)
            nc.vector.tensor_scalar(beta[:], b3[:], 2.0, None, A.mult)

            g1 = f32t("g1", pool=ppool)          # bb*hd
            nc.vector.tensor_tensor(g1[:], bb[:], hd[:], A.mult)
            g1c = f32t("g1c", pool=ppool)        # bb*hd + c
            nc.vector.tensor_tensor(g1c[:], g1[:], c_[:], A.add)
            g2 = f32t("g2", pool=ppool)          # a*hd
            nc.vector.tensor_tensor(g2[:], a_[:], hd[:], A.mult)
            gm = f32t("gm", pool=ppool)
            nc.vector.tensor_tensor(gm[:], g1c[:], g2[:], A.subtract)
            gpl = f32t("gpl", pool=ppool)
            nc.vector.tensor_tensor(gpl[:], g1c[:], g2[:], A.add)
            gamma = f32t("gamma", pool=ppool)    # (bb*hd+c)^2 - (a*hd)^2
            nc.vector.tensor_tensor(gamma[:], gm[:], gpl[:], A.mult)

            # rescaled 16-bit weight columns: alpha*hd^2 (bf16, pairs with
            # bf16 m'^2), beta*hd and gamma (fp16)
            ah1 = f32t("ah1", pool=ppool)        # alpha*hd
            nc.vector.tensor_tensor(ah1[:], alpha[:], hd[:], A.mult)
            ah2 = f32t("ah2", pool=ppool)        # alpha*hd^2
            nc.vector.tensor_tensor(ah2[:], ah1[:], hd[:], A.mult)
            bh = f32t("bh", pool=ppool)          # beta*hd
            nc.vector.tensor_tensor(bh[:], beta[:], hd[:], A.mult)
            w2c = cpool.tile([H, LPC], BF16, tag="w2c")
            nc.vector.tensor_copy(w2c[:], ah2[:])
            w1c = cpool.tile([H, LPC], F16, tag="w1c")
            nc.vector.tensor_copy(w1c[:], bh[:])
            w0c = cpool.tile([H, LPC], F16, tag="w0c")
            nc.vector.tensor_copy(w0c[:], gamma[:])

            # base-term planes (rhs of base matmuls) in bf16 — fp32
            # matmuls are ~8x slower on this path
            A2 = f32t("A2f", pool=ppool)         # a^2
            nc.vector.tensor_tensor(A2[:], a_[:], a_[:], A.mult)
            acen = f32t("acen", pool=ppool)
            nc.vector.tensor_tensor(acen[:], a_[:], cen[:], A.mult)
            A2C2 = f32t("A2C2f", pool=ppool)     # (a*cen)^2
            nc.vector.tensor_tensor(A2C2[:], acen[:], acen[:], A.mult)
            t6 = f32t("t6", pool=ppool)
            nc.vector.tensor_tensor(t6[:], A2[:], cen[:], A.mult)
            M2AC = f32t("M2ACf", pool=ppool)     # -2*a^2*cen
            nc.vector.tensor_scalar(M2AC[:], t6[:], -2.0, None, A.mult)
            A2b = cpool.tile([H, LPC], BF16, tag="A2b")
            nc.vector.tensor_copy(A2b[:], A2[:])
            M2ACb = cpool.tile([H, LPC], BF16, tag="M2ACb")
            nc.vector.tensor_copy(M2ACb[:], M2AC[:])
            A2C2b = cpool.tile([H, LPC], BF16, tag="A2C2b")
            nc.vector.tensor_copy(A2C2b[:], A2C2[:])

            ppool_cm.__exit__(None, None, None)
            l1pool = ctx.enter_context(tc.tile_pool(name="l1", bufs=2))
            mpool = ctx.enter_context(tc.tile_pool(name="m", bufs=2))
            spool = ctx.enter_context(tc.tile_pool(name="s", bufs=2))
            m2pool = ctx.enter_context(tc.tile_pool(name="m2", bufs=2))
            opool = ctx.enter_context(tc.tile_pool(name="outs", bufs=2))
            x2T = cpool.tile([H, B], BF16, tag="x2T")  # x^2 (bf16 lhsT)
            x2f = cpool.tile([H, B], F32, tag="x2f")
            nc.vector.tensor_tensor(x2f[:], xT[:], xT[:], A.mult)
            nc.vector.tensor_copy(x2T[:], x2f[:])
            xTb = cpool.tile([H, B], BF16, tag="xTb")  # x (bf16 lhsT)
            nc.vector.tensor_copy(xTb[:], xT[:])
            ones = cpool.tile([H, 128], BF16, tag="ones")
            nc.gpsimd.memset(ones[:], 1.0)
            x16 = cpool.tile([H, B], F16, tag="x16")   # fp16 x for DVE planes
            nc.vector.tensor_copy(x16[:], xT[:])

            tiles = dict(xTb=xTb, x2T=x2T, ones=ones, invhd=invhd, cod=cod,
                         ncod=ncod, x16=x16, xT=xT, A2b=A2b, M2ACb=M2ACb,
                         A2C2b=A2C2b, w2c=w2c, w1c=w1c, w0c=w0c)
            if repeat > 1:
                with tc.For_i(0, repeat, 1):
                    _run_body(nc, tc, l1pool, mpool, spool, m2pool,
                              pspool, opool, tiles, out_d, ablate)
            else:
                _run_body(nc, tc, l1pool, mpool, spool, m2pool, pspool,
                          opool, tiles, out_d, ablate)

    nc.compile()
    return nc


def _run_body(nc, tc, l1pool, mpool, spool, m2pool, pspool, opool,
              tiles, out_d, ablate=frozenset()):
            xTb, x2T, ones = tiles["xTb"], tiles["x2T"], tiles["ones"]
            invhd, cod, ncod = tiles["invhd"], tiles["cod"], tiles["ncod"]
            x16, xT = tiles["x16"], tiles["xT"]
            A2b, M2ACb, A2C2b = tiles["A2b"], tiles["M2ACb"], tiles["A2C2b"]
            w2c, w1c, w0c = tiles["w2c"], tiles["w1c"], tiles["w0c"]

            # ---- base matmuls into PSUM: dist2 base term (bf16) ----
            psts = []
            for cch in range(NBCH):
                pst = pspool.tile([128, LPC], F32, tag=f"ps{cch}")
                psts.append(pst)
                sl = bass.ts(cch, 128)
                nc.tensor.matmul(pst[:], x2T[:, sl], A2b[:],
                                 start=True, stop=False, skip_group_check=True)
                nc.tensor.matmul(pst[:], xTb[:, sl], M2ACb[:],
                                 start=False, stop=False, skip_group_check=True)
                nc.tensor.matmul(pst[:], ones[:], A2C2b[:],
                                 start=False, stop=False, skip_group_check=True)

            # ---- per-label planes + PE reductions ----
            # NACT of the 8 labels in each group go through ACT Abs; the
            # rest through DVE mult-sub + one grouped sign-clear AND.
            # Interleaving keeps per-group ACT and DVE times matched.
            nact = 0 if "noact" in ablate else ACT_L1_PER_GRP
            ndve = GRP - nact
            for g in range(NGRP):
                l0 = g * GRP
                l1g = l1pool.tile([H, GRP * B], F16, tag="l1g")
                if ndve:
                    tg = l1pool.tile([H, ndve * B], F16, tag="tg")
                    for j in range(ndve):
                        l = l0 + j
                        lsl = slice(l, l + 1)
                        nc.vector.tensor_scalar(tg[:, j * B:(j + 1) * B],
                                                x16[:],
                                                invhd[:, lsl], cod[:, lsl],
                                                A.mult, A.subtract)
                    nc.vector.tensor_scalar(
                        l1g.bitcast(U16)[:, :ndve * B], tg.bitcast(U16)[:],
                        0x7FFF, None, A.bitwise_and)
                for j in range(ndve, GRP):
                    l = l0 + j
                    lsl = slice(l, l + 1)
                    gsl = slice(j * B, (j + 1) * B)
                    nc.scalar.activation(l1g[:, gsl], xT[:], ACT.Abs,
                                         bias=ncod[:, lsl],
                                         scale=invhd[:, lsl])
                m = mpool.tile([H, GRP * B], F16, tag="m")
                nc.vector.tensor_scalar(m[:], l1g[:], 1.0, 0.0,
                                        A.subtract, A.max)
                s = spool.tile([H, GRP * B], F16, tag="s")
                nc.vector.tensor_scalar(s[:], l1g[:], 1.0, None, A.is_gt)
                if "m2" in ablate:
                    m2 = m
                else:
                    m2 = m2pool.tile([H, GRP * B], BF16, tag="m2")
                    nc.scalar.square(m2[:], m[:])   # grouped ACT Square

                if "pe" in ablate:
                    continue
                for j in range(GRP):
                    l = l0 + j
                    lsl = slice(l, l + 1)
                    last = l == LPC - 1
                    for cch in range(NBCH):
                        sl = slice(j * B + cch * 128, j * B + (cch + 1) * 128)
                        pcol = psts[cch][:, lsl]
                        nc.tensor.matmul(pcol, m2[:, sl], w2c[:, lsl],
                                         start=False, stop=False,
                                         skip_group_check=True)
                        nc.tensor.matmul(pcol, m[:, sl], w1c[:, lsl],
                                         start=False, stop=False,
                                         skip_group_check=True)
                        nc.tensor.matmul(pcol, s[:, sl], w0c[:, lsl],
                                         start=False, stop=last,
                                         skip_group_check=True)

            # ---- finalize: out = -sqrt(psum) ----
            for cch in range(NBCH):
                sq = opool.tile([128, LPC], F32, tag="sq")
                nc.scalar.sqrt(sq[:], psts[cch][:])
                o = opool.tile([128, LPC], F32, tag="o")
                nc.vector.tensor_scalar(o[:], sq[:], -1.0, None, A.mult)
                nc.sync.dma_start(out_d.ap()[bass.ts(cch, 128), :], o[:])


_NC_CACHE = None


def _get_nc():
    global _NC_CACHE
    if _NC_CACHE is None:
        _NC_CACHE = build_nc()
    return _NC_CACHE


def kernel(y: np.ndarray, x: np.ndarray) -> np.ndarray:
    y = np.asarray(y, dtype=np.float32)
    x = np.asarray(x, dtype=np.float32)
    assert y.shape == (L, 2 * H) and x.shape == (B, H)

    nc = _get_nc()
    xT = np.ascontiguousarray(x.T)                       # (H, B)
    in_maps = []
    for c in range(N_CORES):
        ys = y[c * LPC:(c + 1) * LPC]
        in_maps.append({
            "xT": xT,
            "mnT": np.ascontiguousarray(ys[:, :H].T),    # (H, LPC)
            "rawT": np.ascontiguousarray(ys[:, H:].T),   # (H, LPC)
        })
    res = bass_utils.run_bass_kernel_spmd(nc, in_maps,
                                          core_ids=list(range(N_CORES)))
    out = np.concatenate([res.results[c]["out"] for c in range(N_CORES)],
                         axis=1)
    return np.ascontiguousarray(out.astype(np.float32))
